# revision 9
# baseline (speedup 1.0000x reference)
"""Bahdanau-attention kernel for Trainium2, SPMD over 8 NeuronCores.

Reference computation (B=64, S=1024, H=1024):
    Wh, We = attn_W[:, :H], attn_W[:, H:]
    h_proj = hidden @ Wh.T                                  # [B, H]
    e_proj[b,s,o] = sum_h enc[s,b,h] * We[o,h]              # [B, S, H]
    energy = tanh(h_proj[:,None,:] + e_proj + attn_b)       # [B, S, H]
    scores = energy @ v_W                                   # [B, S]
    scores = where(mask==0, -1e10, scores)
    out = softmax(scores, axis=1)

Strategy: data-parallel over batch (8 batches per core). Host pre-transposes
inputs so every SBUF tile loads with contiguous 4KB rows and the contraction
dim (h) lands on partitions — no on-chip transposes. All matmuls run as
float32r (full-rate PE).
"""

import numpy as np

import concourse.bass as bass
from concourse import bacc
import concourse.mybir as mybir
import concourse.tile as tile
from concourse.bass_utils import run_bass_kernel_spmd

N_CORES = 8
B, S, H = 64, 1024, 1024
B_LOC = B // N_CORES          # 8 batches per core
NH = H // 128                 # 8 contraction chunks
NO = H // 128                 # 8 output-row tiles
NSH = S // 512                # 2 free-dim halves per s row
FP32 = mybir.dt.float32
FP32R = mybir.dt.float32r
I32 = mybir.dt.int32
AF = mybir.ActivationFunctionType


def build_nc() -> bass.Bass:
    nc = bacc.Bacc()

    encT = nc.declare_dram_parameter("encT", [B_LOC, H, S], FP32R, isOutput=False)
    wT = nc.declare_dram_parameter("wT", [2 * H, H], FP32R, isOutput=False)
    hT = nc.declare_dram_parameter("hT", [H, B_LOC], FP32R, isOutput=False)
    v2d = nc.declare_dram_parameter("v2d", [128, NO], FP32R, isOutput=False)
    b2d = nc.declare_dram_parameter("b2d", [128, NO], FP32, isOutput=False)
    mask = nc.declare_dram_parameter("mask", [B_LOC, S], I32, isOutput=False)
    out = nc.declare_dram_parameter("out", [B_LOC, S], FP32, isOutput=True)

    with tile.TileContext(nc) as tc:
        with (
            tc.tile_pool(name="weights", bufs=1) as wpool,
            tc.tile_pool(name="enc", bufs=2 * NH) as epool,
            tc.tile_pool(name="energy", bufs=4) as enpool,
            tc.tile_pool(name="singles", bufs=1) as singles,
            tc.tile_pool(name="psum", bufs=4, space="PSUM") as psum,
        ):
            # ---- one-time loads -------------------------------------------
            wt = []
            for j in range(2 * NH):
                t = wpool.tile([128, H], FP32R, tag=f"w{j}", name=f"w{j}")
                nc.sync.dma_start(out=t[:], in_=wT[j * 128 : (j + 1) * 128, :])
                wt.append(t)

            hT_t = singles.tile([128, NH, B_LOC], FP32R, tag="hT")
            nc.sync.dma_start(
                out=hT_t[:], in_=hT.rearrange("(c p) b -> p c b", p=128)
            )
            v2d_t = singles.tile([128, NO], FP32R, tag="v2d")
            nc.sync.dma_start(out=v2d_t[:], in_=v2d[:])
            b2d_t = singles.tile([128, NO], FP32, tag="b2d")
            nc.sync.dma_start(out=b2d_t[:], in_=b2d[:])
            mask_i = singles.tile([B_LOC, S], I32, tag="mask_i")
            nc.sync.dma_start(out=mask_i[:], in_=mask[:])

            # ---- c[o, b] = Wh @ hidden.T + attn_b -------------------------
            # c_sb[:, ot*B_LOC + b] is the per-(o_tile, batch) bias column.
            c_sb = singles.tile([128, NO * B_LOC], FP32, tag="c")
            for ot in range(NO):
                c_ps = psum.tile([128, B_LOC], FP32, tag="eps")
                for hc in range(NH):
                    nc.tensor.matmul(
                        c_ps[:],
                        lhsT=wt[hc][:, ot * 128 : (ot + 1) * 128],
                        rhs=hT_t[:, hc, :],
                        start=(hc == 0),
                        stop=(hc == NH - 1),
                    )
                nc.vector.tensor_add(
                    c_sb[:, ot * B_LOC : (ot + 1) * B_LOC],
                    c_ps[:],
                    b2d_t[:, ot : ot + 1].broadcast_to((128, B_LOC)),
                )

            # ---- main loop: e_projT -> tanh -> v-reduce -------------------
            # Engines can only address partition bases {0,32,64,96}, so the
            # per-batch [1, 512] score strips are collected on partition 0
            # and SBUF->SBUF DMAs (which can target any partition) move each
            # strip to its [B_LOC, S] row.
            sc_row = singles.tile([1, B_LOC * S], FP32, tag="sc_row")
            scores = singles.tile([B_LOC, S], FP32, tag="scores")
            for b in range(B_LOC):
                enc_t = []
                for hc in range(NH):
                    t = epool.tile([128, S], FP32R, tag="enc", name=f"enc_{b}_{hc}")
                    nc.sync.dma_start(
                        out=t[:], in_=encT[b, hc * 128 : (hc + 1) * 128, :]
                    )
                    enc_t.append(t)

                sc_ps = [psum.tile([1, 512], FP32, tag="sc", name=f"sc_{b}_{sh}") for sh in range(NSH)]
                for ot in range(NO):
                    for sh in range(NSH):
                        eps = psum.tile([128, 512], FP32, tag="eps", name=f"eps_{b}_{ot}_{sh}")
                        for hc in range(NH):
                            nc.tensor.matmul(
                                eps[:],
                                lhsT=wt[NH + hc][
                                    :, ot * 128 : (ot + 1) * 128
                                ],
                                rhs=enc_t[hc][:, sh * 512 : (sh + 1) * 512].bitcast(
                                    FP32R
                                ),
                                start=(hc == 0),
                                stop=(hc == NH - 1),
                            )
                        en = enpool.tile([128, 512], FP32R, tag="energy", name=f"en_{b}_{ot}_{sh}")
                        nc.scalar.activation(
                            en[:],
                            eps[:],
                            AF.Tanh,
                            bias=c_sb[:, ot * B_LOC + b : ot * B_LOC + b + 1],
                            scale=1.0,
                        )
                        nc.tensor.matmul(
                            sc_ps[sh][:],
                            lhsT=v2d_t[:, ot : ot + 1],
                            rhs=en[:],
                            start=(ot == 0),
                            stop=(ot == NO - 1),
                            skip_group_check=True,
                        )
                for sh in range(NSH):
                    nc.vector.tensor_copy(
                        sc_row[0:1, b * S + sh * 512 : b * S + (sh + 1) * 512],
                        sc_ps[sh][:],
                    )
            for bb in range(B_LOC):
                nc.sync.dma_start(
                    out=scores[bb : bb + 1, :], in_=sc_row[0:1, bb * S : (bb + 1) * S]
                )

            # ---- mask + softmax over s ------------------------------------
            maskf = singles.tile([B_LOC, S], FP32, tag="maskf")
            nc.vector.tensor_copy(maskf[:], mask_i[:])
            big = singles.tile([B_LOC, S], FP32, tag="big")
            # big = mask*1e10 - 1e10  ->  0 where mask==1, -1e10 where mask==0
            nc.vector.tensor_scalar(
                out=big[:],
                in0=maskf[:],
                scalar1=1.0e10,
                scalar2=-1.0e10,
                op0=mybir.AluOpType.mult,
                op1=mybir.AluOpType.add,
            )
            nc.vector.tensor_add(scores[:], scores[:], big[:])
            negmx = singles.tile([B_LOC, 1], FP32, tag="negmx")
            nc.vector.reduce_max(
                negmx[:], scores[:], axis=mybir.AxisListType.X, negate=True
            )
            expo = singles.tile([B_LOC, S], FP32, tag="expo")
            nc.scalar.activation(expo[:], scores[:], AF.Exp, bias=negmx[:], scale=1.0)
            sm = singles.tile([B_LOC, 1], FP32, tag="sm")
            nc.vector.reduce_sum(sm[:], expo[:], axis=mybir.AxisListType.X)
            rec = singles.tile([B_LOC, 1], FP32, tag="rec")
            nc.vector.reciprocal(rec[:], sm[:])
            out_t = singles.tile([B_LOC, S], FP32, tag="out_t")
            nc.vector.tensor_mul(
                out_t[:], expo[:], rec[:].broadcast_to((B_LOC, S))
            )
            nc.sync.dma_start(out=out[:], in_=out_t[:])

    nc.compile()
    return nc


_NC_CACHE = None


def _get_nc():
    global _NC_CACHE
    if _NC_CACHE is None:
        _NC_CACHE = build_nc()
    return _NC_CACHE


def _prep_in_maps(hidden, encoder_outputs, attn_mask, attn_W, attn_b, v_W):
    wT = np.ascontiguousarray(attn_W.T)                       # [2H, H]
    v2d = np.ascontiguousarray(v_W.reshape(NO, 128).T)        # [128, NO]
    b2d = np.ascontiguousarray(attn_b.reshape(NO, 128).T)     # [128, NO]
    in_maps = []
    for i in range(N_CORES):
        sl = slice(i * B_LOC, (i + 1) * B_LOC)
        encT = np.ascontiguousarray(
            encoder_outputs[:, sl, :].transpose(1, 2, 0)      # [B_LOC, H, S]
        )
        in_maps.append(
            dict(
                encT=encT,
                wT=wT,
                hT=np.ascontiguousarray(hidden[sl].T),        # [H, B_LOC]
                v2d=v2d,
                b2d=b2d,
                mask=np.ascontiguousarray(attn_mask[sl]),
            )
        )
    return in_maps


def kernel(hidden, encoder_outputs, attn_mask, attn_W, attn_b, v_W, _run_kwargs=None):
    nc = _get_nc()
    in_maps = _prep_in_maps(hidden, encoder_outputs, attn_mask, attn_W, attn_b, v_W)
    res = run_bass_kernel_spmd(
        nc, in_maps, core_ids=list(range(N_CORES)), **(_run_kwargs or {})
    )
    out = np.concatenate([res.results[i]["out"] for i in range(N_CORES)], axis=0)
    if _run_kwargs:
        kernel.last_result = res
    return out


# revision 12
# speedup vs baseline: 1.0061x; 1.0061x over previous
"""Bahdanau-attention kernel for Trainium2, SPMD over 8 NeuronCores.

Reference computation (B=64, S=1024, H=1024):
    Wh, We = attn_W[:, :H], attn_W[:, H:]
    h_proj = hidden @ Wh.T                                  # [B, H]
    e_proj[b,s,o] = sum_h enc[s,b,h] * We[o,h]              # [B, S, H]
    energy = tanh(h_proj[:,None,:] + e_proj + attn_b)       # [B, S, H]
    scores = energy @ v_W                                   # [B, S]
    scores = where(mask==0, -1e10, scores)
    out = softmax(scores, axis=1)

Strategy: data-parallel over batch (8 batches per core). Host pre-transposes
inputs so every SBUF tile loads with contiguous rows and the contraction dim
(h) lands on partitions — no on-chip data transposes. All matmuls run as
float32r (full-rate PE). Per-batch pipeline: accumulate e_projT[o,s] over 8
K-chunks in PSUM, fused tanh(+h_proj+bias) on ACT while evicting PSUM, then a
[128,1]x[128,512] v-matmul accumulates scores in PSUM across o-tiles. The
softmax runs per batch on partition 0, overlapped with the next batch's
matmuls.
"""

import numpy as np

import concourse.bass as bass
from concourse import bacc
import concourse.mybir as mybir
import concourse.tile as tile
from concourse.bass_utils import run_bass_kernel_spmd
from concourse.masks import make_identity

N_CORES = 8
B, S, H = 64, 1024, 1024
B_LOC = B // N_CORES          # 8 batches per core
NH = H // 128                 # 8 contraction chunks
NO = H // 128                 # 8 output-row tiles
NSH = S // 512                # 2 free-dim halves per s row
FP32 = mybir.dt.float32
FP32R = mybir.dt.float32r
I32 = mybir.dt.int32
AF = mybir.ActivationFunctionType


def build_nc() -> bass.Bass:
    nc = bacc.Bacc()

    encT = nc.declare_dram_parameter("encT", [B_LOC, H, S], FP32R, isOutput=False)
    wT = nc.declare_dram_parameter("wT", [2 * H, H], FP32R, isOutput=False)
    hT = nc.declare_dram_parameter("hT", [H, B_LOC], FP32R, isOutput=False)
    v2d = nc.declare_dram_parameter("v2d", [128, NO], FP32R, isOutput=False)
    bias = nc.declare_dram_parameter("bias", [H], FP32R, isOutput=False)
    mask = nc.declare_dram_parameter("mask", [B_LOC, S], I32, isOutput=False)
    out = nc.declare_dram_parameter("out", [B_LOC, S], FP32, isOutput=True)

    with tile.TileContext(nc) as tc:
        with (
            tc.tile_pool(name="weights", bufs=1) as wpool,
            tc.tile_pool(name="energy", bufs=4) as enpool,
            tc.tile_pool(name="singles", bufs=1) as singles,
            tc.tile_pool(name="strips", bufs=2) as strips,
            tc.tile_pool(name="psum", bufs=4, space="PSUM") as psum,
        ):
            # ---- tiny loads first (DMA queue order follows emission) ------
            hT_t = singles.tile([128, NH, B_LOC], FP32R, tag="hT")
            nc.sync.dma_start(
                out=hT_t[:], in_=hT.rearrange("(c p) b -> p c b", p=128)
            )
            v2d_t = singles.tile([128, NO], FP32R, tag="v2d")
            nc.sync.dma_start(out=v2d_t[:], in_=v2d[:])
            b_row = singles.tile([1, H], FP32R, tag="b_row")
            nc.sync.dma_start(out=b_row[:], in_=bias.rearrange("(a h) -> a h", a=1))
            mask_i = singles.tile([B_LOC, S], I32, tag="mask_i")
            nc.sync.dma_start(out=mask_i[:], in_=mask[:])
            ones_f = singles.tile([1, B_LOC], FP32, tag="ones_f")
            nc.vector.memset(ones_f[:], 1.0)
            ones_t = singles.tile([1, B_LOC], FP32R, tag="ones_t")
            nc.vector.tensor_copy(ones_t[:], ones_f[:])
            id8 = singles.tile([B_LOC, B_LOC], FP32, tag="id8")
            make_identity(nc, id8[:])

            # big[b, s] = 0 where mask==1, -1e10 where mask==0 (exact in f32)
            big = singles.tile([B_LOC, S], FP32, tag="big")
            nc.vector.tensor_copy(big[:], mask_i[:])
            nc.vector.tensor_scalar(
                out=big[:],
                in0=big[:],
                scalar1=1.0e10,
                scalar2=-1.0e10,
                op0=mybir.AluOpType.mult,
                op1=mybir.AluOpType.add,
            )

            # ---- c-phase: cT[b, o] = hidden @ Wh.T + attn_b ---------------
            # Transposed form: hT chunks are the (tiny) stationary operand,
            # WhT chunks stream; the bias folds in as a K=1 rank-1 matmul.
            cT_sb = singles.tile([B_LOC, H], FP32, tag="cT")
            with tc.tile_pool(name="wh", bufs=1) as whpool:
                wh = []
                for j in range(NH):
                    t = whpool.tile([128, H], FP32R, tag=f"wh{j}", name=f"wh{j}")
                    nc.sync.dma_start(out=t[:], in_=wT[j * 128 : (j + 1) * 128, :])
                    wh.append(t)
                for sh2 in range(NSH):
                    cps = psum.tile([B_LOC, 512], FP32, tag="eps", name=f"cps{sh2}")
                    for hc in range(NH):
                        nc.tensor.matmul(
                            cps[:],
                            lhsT=hT_t[:, hc, :],
                            rhs=wh[hc][:, sh2 * 512 : (sh2 + 1) * 512],
                            start=(hc == 0),
                            stop=False,
                        )
                    nc.tensor.matmul(
                        cps[:],
                        lhsT=ones_t[:],
                        rhs=b_row[0:1, sh2 * 512 : (sh2 + 1) * 512],
                        start=False,
                        stop=True,
                    )
                    nc.vector.tensor_copy(
                        cT_sb[:, sh2 * 512 : (sh2 + 1) * 512], cps[:]
                    )

            with tc.tile_pool(name="enc", bufs=2 * NH) as epool:
                # WeT loads interleaved with batch-0 enc chunks: the main
                # matmuls become runnable as soon as this stream lands.
                wt_e = []
                enc_t = {}
                for hc in range(NH):
                    t = wpool.tile([128, H], FP32R, tag=f"we{hc}", name=f"we{hc}")
                    nc.sync.dma_start(
                        out=t[:], in_=wT[(NH + hc) * 128 : (NH + hc + 1) * 128, :]
                    )
                    wt_e.append(t)
                    e = epool.tile([128, S], FP32R, tag="enc", name=f"enc_0_{hc}")
                    nc.sync.dma_start(
                        out=e[:], in_=encT[0, hc * 128 : (hc + 1) * 128, :]
                    )
                    enc_t[(0, hc)] = e

                def load_enc(b):
                    for hc in range(NH):
                        e = epool.tile([128, S], FP32R, tag="enc", name=f"enc_{b}_{hc}")
                        nc.sync.dma_start(
                            out=e[:], in_=encT[b, hc * 128 : (hc + 1) * 128, :]
                        )
                        enc_t[(b, hc)] = e

                # c_sb[:, ot*B_LOC + b] = per-(o_tile, batch) ACT bias column
                c_sb = singles.tile([128, NO * B_LOC], FP32, tag="c")
                for ot in range(NO):
                    tp = psum.tile([128, B_LOC], FP32, tag="eps", name=f"tp{ot}")
                    nc.tensor.transpose(
                        tp[:], cT_sb[0:B_LOC, ot * 128 : (ot + 1) * 128], id8[:]
                    )
                    nc.vector.tensor_copy(
                        c_sb[:, ot * B_LOC : (ot + 1) * B_LOC], tp[:]
                    )

                for b in range(B_LOC):
                    if b + 1 < B_LOC:
                        load_enc(b + 1)

                    sc_ps = [
                        psum.tile([1, 512], FP32, tag="sc", name=f"sc_{b}_{sh}")
                        for sh in range(NSH)
                    ]
                    for ot in range(NO):
                        for sh in range(NSH):
                            eps = psum.tile(
                                [128, 512], FP32, tag="eps", name=f"eps_{b}_{ot}_{sh}"
                            )
                            for hc in range(NH):
                                nc.tensor.matmul(
                                    eps[:],
                                    lhsT=wt_e[hc][:, ot * 128 : (ot + 1) * 128],
                                    rhs=enc_t[(b, hc)][:, sh * 512 : (sh + 1) * 512],
                                    start=(hc == 0),
                                    stop=(hc == NH - 1),
                                )
                            en = enpool.tile(
                                [128, 512], FP32R, tag="energy",
                                name=f"en_{b}_{ot}_{sh}",
                            )
                            nc.scalar.activation(
                                en[:],
                                eps[:],
                                AF.Tanh,
                                bias=c_sb[:, ot * B_LOC + b : ot * B_LOC + b + 1],
                                scale=1.0,
                            )
                            nc.tensor.matmul(
                                sc_ps[sh][:],
                                lhsT=v2d_t[:, ot : ot + 1],
                                rhs=en[:],
                                start=(ot == 0),
                                stop=(ot == NO - 1),
                                skip_group_check=True,
                            )

                    # ---- per-batch mask + softmax on partition 0 ----------
                    # Engines can only address partition bases {0,32,64,96};
                    # DMA moves row b of `big`/`out` to/from partition 0.
                    big_s = strips.tile([1, S], FP32, tag="big_s", name=f"bigs{b}")
                    nc.sync.dma_start(out=big_s[:], in_=big[b : b + 1, :])
                    sc_s = strips.tile([1, S], FP32, tag="sc_s", name=f"scs{b}")
                    for sh in range(NSH):
                        nc.vector.tensor_copy(
                            sc_s[0:1, sh * 512 : (sh + 1) * 512], sc_ps[sh][:]
                        )
                    nc.vector.tensor_add(sc_s[:], sc_s[:], big_s[:])
                    negmx = strips.tile([1, 1], FP32, tag="negmx", name=f"nmx{b}")
                    nc.vector.reduce_max(
                        negmx[:], sc_s[:], axis=mybir.AxisListType.X, negate=True
                    )
                    ex_s = strips.tile([1, S], FP32, tag="ex_s", name=f"exs{b}")
                    nc.scalar.activation(
                        ex_s[:], sc_s[:], AF.Exp, bias=negmx[:], scale=1.0
                    )
                    sm = strips.tile([1, 1], FP32, tag="sm", name=f"sm{b}")
                    nc.vector.reduce_sum(sm[:], ex_s[:], axis=mybir.AxisListType.X)
                    rec = strips.tile([1, 1], FP32, tag="rec", name=f"rec{b}")
                    nc.vector.reciprocal(rec[:], sm[:])
                    o_s = strips.tile([1, S], FP32, tag="o_s", name=f"os{b}")
                    nc.vector.tensor_mul(
                        o_s[:], ex_s[:], rec[:].broadcast_to((1, S))
                    )
                    nc.sync.dma_start(out=out[b : b + 1, :], in_=o_s[:])

    nc.compile()
    return nc


_NC_CACHE = None


def _get_nc():
    global _NC_CACHE
    if _NC_CACHE is None:
        _NC_CACHE = build_nc()
    return _NC_CACHE


def _prep_in_maps(hidden, encoder_outputs, attn_mask, attn_W, attn_b, v_W):
    wT = np.ascontiguousarray(attn_W.T)                       # [2H, H]
    v2d = np.ascontiguousarray(v_W.reshape(NO, 128).T)        # [128, NO]
    in_maps = []
    for i in range(N_CORES):
        sl = slice(i * B_LOC, (i + 1) * B_LOC)
        encT = np.ascontiguousarray(
            encoder_outputs[:, sl, :].transpose(1, 2, 0)      # [B_LOC, H, S]
        )
        in_maps.append(
            dict(
                encT=encT,
                wT=wT,
                hT=np.ascontiguousarray(hidden[sl].T),        # [H, B_LOC]
                v2d=v2d,
                bias=np.ascontiguousarray(attn_b),
                mask=np.ascontiguousarray(attn_mask[sl]),
            )
        )
    return in_maps


def kernel(hidden, encoder_outputs, attn_mask, attn_W, attn_b, v_W, _run_kwargs=None):
    nc = _get_nc()
    in_maps = _prep_in_maps(hidden, encoder_outputs, attn_mask, attn_W, attn_b, v_W)
    res = run_bass_kernel_spmd(
        nc, in_maps, core_ids=list(range(N_CORES)), **(_run_kwargs or {})
    )
    out = np.concatenate([res.results[i]["out"] for i in range(N_CORES)], axis=0)
    if _run_kwargs:
        kernel.last_result = res
    return out


# revision 14
# speedup vs baseline: 1.0168x; 1.0106x over previous
"""Bahdanau-attention kernel for Trainium2, SPMD over 8 NeuronCores.

Reference computation (B=64, S=1024, H=1024):
    Wh, We = attn_W[:, :H], attn_W[:, H:]
    h_proj = hidden @ Wh.T                                  # [B, H]
    e_proj[b,s,o] = sum_h enc[s,b,h] * We[o,h]              # [B, S, H]
    energy = tanh(h_proj[:,None,:] + e_proj + attn_b)       # [B, S, H]
    scores = energy @ v_W                                   # [B, S]
    scores = where(mask==0, -1e10, scores)
    out = softmax(scores, axis=1)

Strategy: data-parallel over batch (8 batches per core). The host pre-
transposes inputs so every SBUF tile loads with contiguous rows and the
contraction dim (h) lands on partitions — no on-chip data transposes — and
converts matmul operands to bf16 (halves HBM traffic and SBUF footprint;
PSUM accumulation stays fp32). Batches are processed in pairs so each
stationary weight tile feeds 4 matmuls. Per (o-tile): accumulate
e_projT[o,s] over 8 K-chunks in PSUM, fused tanh(+h_proj+bias) on ACT while
evicting PSUM, then [128,1]x[128,512] v-matmuls accumulate scores in PSUM
across o-tiles. The mask+softmax runs per batch on partition 0, overlapped
with the next batches' matmuls.
"""

import numpy as np
import ml_dtypes

import concourse.bass as bass
from concourse import bacc
import concourse.mybir as mybir
import concourse.tile as tile
from concourse.bass_utils import run_bass_kernel_spmd
from concourse.masks import make_identity

N_CORES = 8
B, S, H = 64, 1024, 1024
B_LOC = B // N_CORES          # 8 batches per core
NH = H // 128                 # 8 contraction chunks
NO = H // 128                 # 8 output-row tiles
NSH = S // 512                # 2 free-dim halves per s row
FP32 = mybir.dt.float32
BF16 = mybir.dt.bfloat16
I32 = mybir.dt.int32
AF = mybir.ActivationFunctionType
BF16_NP = ml_dtypes.bfloat16

GROUPS = [[0, 1], [2, 3], [4, 5], [6, 7]]


def build_nc() -> bass.Bass:
    nc = bacc.Bacc()

    encT = nc.declare_dram_parameter("encT", [B_LOC, H, S], BF16, isOutput=False)
    wT = nc.declare_dram_parameter("wT", [2 * H, H], BF16, isOutput=False)
    hT = nc.declare_dram_parameter("hT", [H, B_LOC], BF16, isOutput=False)
    v2d = nc.declare_dram_parameter("v2d", [128, NO], BF16, isOutput=False)
    bias = nc.declare_dram_parameter("bias", [H], BF16, isOutput=False)
    mask = nc.declare_dram_parameter("mask", [B_LOC, S], I32, isOutput=False)
    out = nc.declare_dram_parameter("out", [B_LOC, S], FP32, isOutput=True)

    with tile.TileContext(nc) as tc:
        with (
            tc.tile_pool(name="weights", bufs=1) as wpool,
            tc.tile_pool(name="energy", bufs=6) as enpool,
            tc.tile_pool(name="singles", bufs=1) as singles,
            tc.tile_pool(name="strips", bufs=2) as strips,
            tc.tile_pool(name="psum", bufs=4, space="PSUM") as psum,
        ):
            # ---- tiny loads first (DMA queue order follows emission) ------
            hT_t = singles.tile([128, NH, B_LOC], BF16, tag="hT")
            nc.sync.dma_start(
                out=hT_t[:], in_=hT.rearrange("(c p) b -> p c b", p=128)
            )
            v2d_t = singles.tile([128, NO], BF16, tag="v2d")
            nc.sync.dma_start(out=v2d_t[:], in_=v2d[:])
            b_row = singles.tile([1, H], BF16, tag="b_row")
            nc.sync.dma_start(out=b_row[:], in_=bias.rearrange("(a h) -> a h", a=1))
            mask_i = singles.tile([B_LOC, S], I32, tag="mask_i")
            nc.sync.dma_start(out=mask_i[:], in_=mask[:])
            ones_t = singles.tile([1, B_LOC], BF16, tag="ones_t")
            nc.vector.memset(ones_t[:], 1.0)
            id8 = singles.tile([B_LOC, B_LOC], FP32, tag="id8")
            make_identity(nc, id8[:])

            # big[b, s] = 0 where mask==1, -1e10 where mask==0 (exact in f32)
            big = singles.tile([B_LOC, S], FP32, tag="big")
            nc.vector.tensor_copy(big[:], mask_i[:])
            nc.vector.tensor_scalar(
                out=big[:],
                in0=big[:],
                scalar1=1.0e10,
                scalar2=-1.0e10,
                op0=mybir.AluOpType.mult,
                op1=mybir.AluOpType.add,
            )

            # ---- c-phase: cT[b, o] = hidden @ Wh.T + attn_b ---------------
            # Transposed form: hT chunks are the (tiny) stationary operand,
            # WhT chunks stream; the bias folds in as a K=1 rank-1 matmul.
            cT_sb = singles.tile([B_LOC, H], FP32, tag="cT")
            with tc.tile_pool(name="wh", bufs=1) as whpool:
                wh = []
                for j in range(NH):
                    t = whpool.tile([128, H], BF16, tag=f"wh{j}", name=f"wh{j}")
                    nc.sync.dma_start(out=t[:], in_=wT[j * 128 : (j + 1) * 128, :])
                    wh.append(t)
                for sh2 in range(NSH):
                    cps = psum.tile([B_LOC, 512], FP32, tag="eps", name=f"cps{sh2}")
                    for hc in range(NH):
                        nc.tensor.matmul(
                            cps[:],
                            lhsT=hT_t[:, hc, :],
                            rhs=wh[hc][:, sh2 * 512 : (sh2 + 1) * 512],
                            start=(hc == 0),
                            stop=False,
                        )
                    nc.tensor.matmul(
                        cps[:],
                        lhsT=ones_t[:],
                        rhs=b_row[0:1, sh2 * 512 : (sh2 + 1) * 512],
                        start=False,
                        stop=True,
                    )
                    nc.vector.tensor_copy(
                        cT_sb[:, sh2 * 512 : (sh2 + 1) * 512], cps[:]
                    )

            with tc.tile_pool(name="enc", bufs=4 * NH) as epool:
                # WeT loads interleaved with group-0 enc chunks: the main
                # matmuls become runnable as soon as this stream lands.
                wt_e = []
                enc_t = {}

                def load_enc(b):
                    for hc in range(NH):
                        e = epool.tile(
                            [128, S], BF16, tag="enc", name=f"enc_{b}_{hc}"
                        )
                        nc.sync.dma_start(
                            out=e[:], in_=encT[b, hc * 128 : (hc + 1) * 128, :]
                        )
                        enc_t[(b, hc)] = e

                for hc in range(NH):
                    t = wpool.tile([128, H], BF16, tag=f"we{hc}", name=f"we{hc}")
                    nc.sync.dma_start(
                        out=t[:], in_=wT[(NH + hc) * 128 : (NH + hc + 1) * 128, :]
                    )
                    wt_e.append(t)
                    for b in GROUPS[0]:
                        e = epool.tile(
                            [128, S], BF16, tag="enc", name=f"enc_{b}_{hc}"
                        )
                        nc.sync.dma_start(
                            out=e[:], in_=encT[b, hc * 128 : (hc + 1) * 128, :]
                        )
                        enc_t[(b, hc)] = e

                # c_sb[:, ot*B_LOC + b] = per-(o_tile, batch) ACT bias column
                c_sb = singles.tile([128, NO * B_LOC], FP32, tag="c")
                for ot in range(NO):
                    tp = psum.tile([128, B_LOC], FP32, tag="eps", name=f"tp{ot}")
                    nc.tensor.transpose(
                        tp[:], cT_sb[0:B_LOC, ot * 128 : (ot + 1) * 128], id8[:]
                    )
                    nc.vector.tensor_copy(
                        c_sb[:, ot * B_LOC : (ot + 1) * B_LOC], tp[:]
                    )

                for gi, grp in enumerate(GROUPS):
                    if gi + 1 < len(GROUPS):
                        for nb in GROUPS[gi + 1]:
                            load_enc(nb)

                    sc_ps = {
                        (b, sh): psum.tile(
                            [1, 512], FP32, tag="sc", name=f"sc_{b}_{sh}"
                        )
                        for b in grp
                        for sh in range(NSH)
                    }
                    for ot in range(NO):
                        epss = {}
                        for b in grp:
                            for sh in range(NSH):
                                epss[(b, sh)] = psum.tile(
                                    [128, 512], FP32, tag="eps",
                                    name=f"eps_{b}_{ot}_{sh}",
                                )
                        for hc in range(NH):
                            for b in grp:
                                for sh in range(NSH):
                                    nc.tensor.matmul(
                                        epss[(b, sh)][:],
                                        lhsT=wt_e[hc][:, ot * 128 : (ot + 1) * 128],
                                        rhs=enc_t[(b, hc)][
                                            :, sh * 512 : (sh + 1) * 512
                                        ],
                                        start=(hc == 0),
                                        stop=(hc == NH - 1),
                                    )
                        ens = {}
                        for b in grp:
                            for sh in range(NSH):
                                en = enpool.tile(
                                    [128, 512], BF16, tag="energy",
                                    name=f"en_{b}_{ot}_{sh}",
                                )
                                nc.scalar.activation(
                                    en[:],
                                    epss[(b, sh)][:],
                                    AF.Tanh,
                                    bias=c_sb[
                                        :, ot * B_LOC + b : ot * B_LOC + b + 1
                                    ],
                                    scale=1.0,
                                )
                                ens[(b, sh)] = en
                        for b in grp:
                            for sh in range(NSH):
                                nc.tensor.matmul(
                                    sc_ps[(b, sh)][:],
                                    lhsT=v2d_t[:, ot : ot + 1],
                                    rhs=ens[(b, sh)][:],
                                    start=(ot == 0),
                                    stop=(ot == NO - 1),
                                    skip_group_check=True,
                                )

                    # ---- per-batch mask + softmax on partition 0 ----------
                    # Engines can only address partition bases {0,32,64,96};
                    # DMA moves row b of `big`/`out` to/from partition 0.
                    for b in grp:
                        big_s = strips.tile(
                            [1, S], FP32, tag="big_s", name=f"bigs{b}"
                        )
                        nc.sync.dma_start(out=big_s[:], in_=big[b : b + 1, :])
                        sc_s = strips.tile([1, S], FP32, tag="sc_s", name=f"scs{b}")
                        for sh in range(NSH):
                            nc.vector.tensor_copy(
                                sc_s[0:1, sh * 512 : (sh + 1) * 512],
                                sc_ps[(b, sh)][:],
                            )
                        nc.vector.tensor_add(sc_s[:], sc_s[:], big_s[:])
                        negmx = strips.tile(
                            [1, 1], FP32, tag="negmx", name=f"nmx{b}"
                        )
                        nc.vector.reduce_max(
                            negmx[:], sc_s[:], axis=mybir.AxisListType.X, negate=True
                        )
                        ex_s = strips.tile([1, S], FP32, tag="ex_s", name=f"exs{b}")
                        nc.scalar.activation(
                            ex_s[:], sc_s[:], AF.Exp, bias=negmx[:], scale=1.0
                        )
                        sm = strips.tile([1, 1], FP32, tag="sm", name=f"sm{b}")
                        nc.vector.reduce_sum(
                            sm[:], ex_s[:], axis=mybir.AxisListType.X
                        )
                        rec = strips.tile([1, 1], FP32, tag="rec", name=f"rec{b}")
                        nc.vector.reciprocal(rec[:], sm[:])
                        o_s = strips.tile([1, S], FP32, tag="o_s", name=f"os{b}")
                        nc.vector.tensor_mul(
                            o_s[:], ex_s[:], rec[:].broadcast_to((1, S))
                        )
                        nc.sync.dma_start(out=out[b : b + 1, :], in_=o_s[:])

    nc.compile()
    return nc


_NC_CACHE = None


def _get_nc():
    global _NC_CACHE
    if _NC_CACHE is None:
        _NC_CACHE = build_nc()
    return _NC_CACHE


def _prep_in_maps(hidden, encoder_outputs, attn_mask, attn_W, attn_b, v_W):
    wT = np.ascontiguousarray(attn_W.T).astype(BF16_NP)       # [2H, H]
    v2d = np.ascontiguousarray(v_W.reshape(NO, 128).T).astype(BF16_NP)
    bias_bf = np.ascontiguousarray(attn_b).astype(BF16_NP)
    in_maps = []
    for i in range(N_CORES):
        sl = slice(i * B_LOC, (i + 1) * B_LOC)
        encT = np.ascontiguousarray(
            encoder_outputs[:, sl, :].transpose(1, 2, 0)      # [B_LOC, H, S]
        ).astype(BF16_NP)
        in_maps.append(
            dict(
                encT=encT,
                wT=wT,
                hT=np.ascontiguousarray(hidden[sl].T).astype(BF16_NP),
                v2d=v2d,
                bias=bias_bf,
                mask=np.ascontiguousarray(attn_mask[sl]),
            )
        )
    return in_maps


def kernel(hidden, encoder_outputs, attn_mask, attn_W, attn_b, v_W, _run_kwargs=None):
    nc = _get_nc()
    in_maps = _prep_in_maps(hidden, encoder_outputs, attn_mask, attn_W, attn_b, v_W)
    res = run_bass_kernel_spmd(
        nc, in_maps, core_ids=list(range(N_CORES)), **(_run_kwargs or {})
    )
    out = np.concatenate([res.results[i]["out"] for i in range(N_CORES)], axis=0)
    if _run_kwargs:
        kernel.last_result = res
    return out
